# revision 16
# baseline (speedup 1.0000x reference)
"""LocallyConnected2dV2 Trainium2 kernel (bf16, dual-engine DMA issue).

Math: out[b, a, bp] = sum_{k,l} xpad[b, 5a+k, 5bp+l] * kw[a, bp, k, l] + bias[a, bp]
where xpad is x zero-padded by 2, kw is extracted from the sparse-structured
W[17424, 625] (100 nonzeros per column at statically known positions).

Strategy (8 cores, data-parallel over batch, 128 images/core):
  - Host: transpose each core's x shard to [col j', row r, batch b] so the
    contraction dim (image columns) lands on SBUF partitions; cast bf16.
  - Host: compact W into per-image-row banded blocks Wh[r, j', 50] (bf16).
  - Device: for each output-row group g (5 groups x 5 a's = 125 psum cols),
    accumulate over the ~30 contributing image rows:
      psum[b, nk] += xT_row[j', b].T @ Wh_row[j', 50]   (bf16 matmuls, fp32 psum)
    Bias enters as the last matmul of each group via a K=1 ones-vector matmul.
  - DMA: 8 combined x+w chunks, issue alternating between the two HWDGE
    engines (sync/scalar) to halve serialized descriptor-gen time. Output
    streams out per group as bf16 to a group-major DRAM buffer.
"""

import numpy as np
import ml_dtypes

BF16 = ml_dtypes.bfloat16

B = 1024
R = 128           # image rows = cols
NCORES = 8
BS = B // NCORES  # 128 batch per core
NK = 625
WP = 132
NG = 5            # output-row groups (5 a's each)
GW = 125          # psum cols per group
CHUNK = 16        # image rows per DMA chunk
NCH = R // CHUNK


def _a0_of_row(r):
    return min(max((r - 3) // 5, 0), 23)


def _group_rows(g):
    return range(max(0, 25 * g - 2), min(R - 1, 25 * g + 27) + 1)


def _row_parts(r, g):
    """Matmul pieces row r contributes to group g.

    Returns list of (psum_col, n_cols, w_col): psum slice [psum_col, +n),
    weight slice [w_col, +n) within the row's 50-wide weight block.
    """
    a0 = _a0_of_row(r)
    lo = 5 * g
    if a0 >= lo and a0 + 1 < lo + 5:
        return [((a0 - lo) * 25, 50, 0)]
    parts = []
    for ai, a in ((0, a0), (1, a0 + 1)):
        if lo <= a < lo + 5:
            parts.append(((a - lo) * 25, 25, ai * 25))
    return parts


def prep_weights(W, bias):
    """W [17424, 625], bias [25,25] -> wt [128, 128*50] ([j'][r, c] layout),
    bs [1, 625] (fp32; cast to bf16 at pack time)."""
    W = np.asarray(W, np.float32)
    i = np.arange(NK)
    si = (i // 25) * 5
    sj = (i % 25) * 5
    rows = ((si[:, None, None] + np.arange(10)[None, :, None]) * WP
            + sj[:, None, None] + np.arange(10)[None, None, :])
    kw = W[rows.reshape(NK, 100), i[:, None]].reshape(25, 25, 10, 10)

    r = np.arange(R)[:, None, None]
    jp = np.arange(R)[None, :, None]
    c = np.arange(50)[None, None, :]
    ai = c // 25
    bp = c % 25
    a = np.clip((r - 3) // 5, 0, 23) + ai
    k = r + 2 - 5 * a
    l = jp + 2 - 5 * bp
    valid = (k >= 0) & (k < 10) & (l >= 0) & (l < 10)
    Wh = np.where(valid, kw[a, bp, np.clip(k, 0, 9), np.clip(l, 0, 9)], 0.0)
    Wh = Wh.astype(np.float32)                       # [r, j', 50]
    wt = np.ascontiguousarray(Wh.transpose(1, 0, 2)).reshape(R, R * 50)
    bs = np.ascontiguousarray(np.asarray(bias, np.float32).reshape(1, NK))
    return wt, bs


CW = CHUNK * BS + CHUNK * 50   # combined x+w free cols per chunk


def _build_nc():
    import concourse.bass as bass
    import concourse.mybir as mybir
    import concourse.tile as tile
    from concourse import bacc

    bf16 = mybir.dt.bfloat16
    nc = bacc.Bacc("TRN2", target_bir_lowering=False, debug=False)
    xw = nc.dram_tensor("xw", [R, NCH * CW], bf16, kind="ExternalInput").ap()
    aux = nc.dram_tensor("aux", [1, NK + BS], bf16, kind="ExternalInput").ap()
    out = nc.dram_tensor("out", [NG * BS, GW], bf16, kind="ExternalOutput").ap()

    # Chunks 0/1 stream via raw-bass DMAs issued BEFORE the TileContext so
    # their transfers overlap the ~1.2us context-entry prologue. Their
    # consumers are gated by two wait_ge instructions pinned at the head of
    # the in-order PE stream.
    e0 = nc.alloc_sbuf_tensor("e0", [R, CW], bf16)
    e1 = nc.alloc_sbuf_tensor("e1", [R, CW], bf16)
    s0 = nc.alloc_semaphore("early0")
    s1 = nc.alloc_semaphore("early1")
    nc.sync.dma_start(e0.ap(), xw[:, 0:CW]).then_inc(s0, 16)
    nc.scalar.dma_start(e1.ap(), xw[:, CW:2 * CW]).then_inc(s1, 16)

    with tile.TileContext(nc) as tc:
        with (
            tc.tile_pool(name="xw", bufs=1) as xw_pool,
            tc.tile_pool(name="small", bufs=1) as small,
            tc.tile_pool(name="ps", bufs=5, space=bass.MemorySpace.PSUM) as ps_pool,
            tc.tile_pool(name="dps", bufs=1, space=bass.MemorySpace.PSUM) as dps_pool,
            tc.tile_pool(name="ob", bufs=1) as ob_pool,
        ):
            aux_t = small.tile([1, NK + BS], bf16, tag="aux")
            nc.scalar.dma_start(aux_t[:], aux[:])
            bias_t = aux_t[:, 0:NK]
            ones_t = aux_t[:, NK:NK + BS]

            ch = [e0.ap(), e1.ap()]
            for ic in range(2, NCH):
                t = xw_pool.tile([R, CW], bf16, tag=f"c{ic}")
                eng = nc.sync if ic % 2 == 0 else nc.scalar
                eng.dma_start(t[:], xw[:, ic * CW:(ic + 1) * CW])
                ch.append(t)

            # two tiny constant matmuls head the PE stream; the early-chunk
            # waits attach to them post-scheduling. A waiting instruction
            # stalls NX dispatch, gating every later Ldweights/Matmult.
            one_bf = nc.const_aps.aps[(mybir.dt.bfloat16, 1.0)]
            dps = dps_pool.tile([1, 1], mybir.dt.float32, tag="dummy")
            gate0 = nc.tensor.matmul(dps[0:1, 0:1], one_bf[0:1, 0:1],
                                     one_bf[0:1, 0:1], start=True, stop=True,
                                     skip_group_check=True)
            gate1 = nc.tensor.matmul(dps[0:1, 0:1], one_bf[0:1, 0:1],
                                     one_bf[0:1, 0:1], start=True, stop=True,
                                     skip_group_check=True)

            mm_insts = []        # (inst, row) in PE program order
            out_sb = ob_pool.tile([BS, NK], bf16, tag="osb")
            for g in range(NG):
                ps = ps_pool.tile([BS, GW], mybir.dt.float32)
                mms = []
                for r in _group_rows(g):
                    ct = ch[r // CHUNK]
                    lhsT = ct[:, (r % CHUNK) * BS:(r % CHUNK + 1) * BS]
                    wb = CHUNK * BS + (r % CHUNK) * 50
                    for (pc, n, wc) in _row_parts(r, g):
                        mms.append((ps[:, pc:pc + n], lhsT,
                                    ct[:, wb + wc:wb + wc + n], r))
                # bias enters last so aux stays off the group-start path
                mms.append((ps[:, 0:GW], ones_t,
                            bias_t[:, g * GW:(g + 1) * GW], -1))
                last = len(mms) - 1
                for idx, (o, lh, rh, r) in enumerate(mms):
                    inst = nc.tensor.matmul(o, lh, rh,
                                            start=(idx == 0), stop=(idx == last))
                    mm_insts.append((inst, r))
                nc.vector.tensor_copy(
                    out_sb[:, g * GW:(g + 1) * GW], ps[:])
                nc.scalar.dma_start(out[g * BS:(g + 1) * BS, :],
                                    out_sb[:, g * GW:(g + 1) * GW])
    # Attach the early-chunk waits post-scheduling: the Tile simulator does
    # not model the pre-context DMAs and would report a false deadlock. PE is
    # in-order, so a wait on matmul i gates all later matmuls; if an
    # instruction's wait slots are full, fall back to an earlier one.
    def attach(sem, start_idx):
        for i in range(start_idx, -1, -1):
            try:
                mm_insts[i][0]._wait_ge(sem, 16)
                return
            except AssertionError:
                continue
        raise RuntimeError("no wait slot available")

    gate0._wait_ge(s0, 16)
    gate1._wait_ge(s1, 16)
    nc.compile()
    return nc


_NC_CACHE = []


def _get_nc():
    if not _NC_CACHE:
        _NC_CACHE.append(_build_nc())
    return _NC_CACHE[0]


def make_in_maps(x, W, bias):
    x = np.asarray(x, np.float32)
    wt, bsv = prep_weights(W, bias)
    wt16 = wt.astype(BF16)
    auxv = np.concatenate(
        [bsv.astype(BF16), np.ones((1, BS), BF16)], axis=1)
    in_maps = []
    for c in range(NCORES):
        xc = x[c * BS:(c + 1) * BS]                      # [b, r, j']
        xtv = np.ascontiguousarray(
            xc.transpose(2, 1, 0)).astype(BF16).reshape(R, R * BS)
        parts = []
        for ic in range(NCH):
            parts.append(xtv[:, ic * CHUNK * BS:(ic + 1) * CHUNK * BS])
            parts.append(wt16[:, ic * CHUNK * 50:(ic + 1) * CHUNK * 50])
        xwv = np.ascontiguousarray(np.concatenate(parts, axis=1))
        in_maps.append({"xw": xwv, "aux": auxv})
    return in_maps


def run(x, W, bias, trace=False, **kw):
    from concourse import bass_utils
    nc = _get_nc()
    res = bass_utils.run_bass_kernel_spmd(
        nc, make_in_maps(x, W, bias), list(range(NCORES)), trace=trace, **kw)
    outs = []
    for c in range(NCORES):
        o = np.asarray(res.results[c]["out"])            # [NG*BS, GW] bf16
        o = o.reshape(NG, BS, GW).transpose(1, 0, 2)     # [BS, NG, GW]
        outs.append(o.reshape(BS, 25, 25).astype(np.float32))
    return np.concatenate(outs, axis=0), res


def kernel(**inputs):
    out, _ = run(inputs["x"], inputs["W"], inputs["bias"])
    return out
